# revision 47
# baseline (speedup 1.0000x reference)
"""Multi-head self-attention TRN2 Bass kernel.

Problem: x[2, 2048, 1024], 16 heads x 64 dim, fp32.
Sharding: 8 cores = 2 batches x 4 head-groups (4 heads each).
Each core computes its batch's partial output (its 4 heads through
QKV -> attention -> output projection rows); host sums the 4 partials
per batch and adds bo.

Single fully-pipelined stream (no separate projection phase):
  - warmup matmuls on a memset tile from t~0 keep the PE p-state/HAM
    clock ramping while the input DMAs land (~9-12us lead-in).
  - minimal prologue: kT m=0 seq-chunk 0 + qT chunk 0 m=0, then the
    attention master loop starts immediately (~13us vs ~49us before).
  - ALL remaining projection work (K m0 jc1-3, K m1, Q c0 m1, Q c1-3,
    V st0-15) runs as deadline-scheduled fillers inside the attention
    stream: forced just-in-time by data deadlines, plus linear pacing
    so the late (ACT-bound) units carry real work instead of dummies.
  - scores computed TRANSPOSED per head-pair (two K=64 matmuls); exp on
    ACT -> A^T bf16 rolling buffer; AV lagged one kt slot behind exp so
    the PE never waits on the ACT engine.
  - normalize via fast-reciprocal + rank-1 PE broadcast; out_proj
    (K=128 stacked head-pairs) enqueued as fillers into the next unit.
  - output partials DMA'd out as bf16 (halves output traffic; host
    accumulates in f32 and adds bo).
"""

import numpy as np

S = 2048          # sequence length per batch
H = 1024          # hidden
G = 256           # head-group width (4 heads x 64)
HD = 65           # V' columns per head (64 + ones)
NHL = 4           # heads per core
N_CORES = 8

_CACHE = {}


def _build():
    if "nc" in _CACHE:
        return _CACHE["nc"]

    import concourse.bass as bass
    import concourse.mybir as mybir
    import concourse.tile as tile
    from concourse import bacc
    from concourse.tile_rust import add_dep_helper

    f32 = mybir.dt.float32
    bf16 = mybir.dt.bfloat16
    EXP = mybir.ActivationFunctionType.Exp

    nc = bacc.Bacc("TRN2", target_bir_lowering=False, debug=False,
                   num_devices=N_CORES)

    xt_in = nc.dram_tensor("xt", [4, 128, 8, 512], bf16, kind="ExternalInput")
    wq_in = nc.dram_tensor("wq", [H, G], bf16, kind="ExternalInput")
    wk_in = nc.dram_tensor("wk", [H, G], bf16, kind="ExternalInput")
    wv_in = nc.dram_tensor("wv", [H, G], bf16, kind="ExternalInput")
    bq_in = nc.dram_tensor("bq", [G, 1], f32, kind="ExternalInput")
    bk_in = nc.dram_tensor("bk", [G, 1], f32, kind="ExternalInput")
    bv_in = nc.dram_tensor("bv", [G], f32, kind="ExternalInput")
    wo_in = nc.dram_tensor("wo", [NHL, 64, H], bf16, kind="ExternalInput")
    out_d = nc.dram_tensor("out", [S, H], bf16, kind="ExternalOutput")

    with tile.TileContext(nc) as tc:
        with (
            tc.tile_pool(name="persist", bufs=1) as persist,
            tc.tile_pool(name="at_roll", bufs=2) as at_pool,
            tc.tile_pool(name="outP", bufs=4) as op_pool,
            tc.tile_pool(name="tmpo", bufs=1) as tmpo_pool,
            tc.tile_pool(name="sums", bufs=4) as sums_pool,
            tc.tile_pool(name="osb", bufs=4) as osb_pool,
            tc.tile_pool(name="ps_s", bufs=2, space="PSUM") as ps_s_pool,
            tc.tile_pool(name="ps_av", bufs=2, space="PSUM") as ps_av_pool,
            tc.tile_pool(name="ps_op", bufs=1, space="PSUM") as ps_op_pool,
        ):
            qT = persist.tile([128, 2, S], bf16)     # [qd, m, s]
            kT = persist.tile([128, 2, S], bf16)
            vp = persist.tile([128, 16, NHL, HD], bf16)  # [s-part, st, h, col]
            bq_sb = persist.tile([128, 2, 1], f32)
            bk_sb = persist.tile([128, 2, 1], f32)
            bv_bc = persist.tile([128, G], f32)
            wo_pr = persist.tile([128, 2, H], bf16)
            ones64 = persist.tile([1, 64], bf16)
            warm = persist.tile([128, 512], bf16)

            wq_sb = persist.tile([128, 8, G], bf16)
            wk_sb = persist.tile([128, 8, G], bf16)
            wv_sb = persist.tile([128, 8, G], bf16)

            # warmup scratch is memset (no DMA dependency) so the PE can
            # start ramping its clock immediately
            nc.gpsimd.memset(warm, 0.0)

            # ---- input DMAs; the critical first wave (wk, wq, x0) gets
            # the full DMA bandwidth — everything bulky that is needed
            # later (x1-3, wv, wo) is dep-chained behind it ----
            xTc = [persist.tile([128, 8, 512], bf16, name=f"xT_{jc}")
                   for jc in range(4)]
            nc.sync.dma_start(
                out=wk_sb, in_=wk_in.ap().rearrange("(t p) d -> p t d", p=128))
            nc.sync.dma_start(
                out=wq_sb, in_=wq_in.ap().rearrange("(t p) d -> p t d", p=128))
            x_dmas = [nc.sync.dma_start(out=xTc[0], in_=xt_in.ap()[0])]
            for jc in range(1, 4):
                dma = nc.sync.dma_start(out=xTc[jc], in_=xt_in.ap()[jc])
                add_dep_helper(dma.ins, x_dmas[-1].ins,
                               reason="serialize x chunk loads")
                x_dmas.append(dma)
            nc.sync.dma_start(
                out=wv_sb, in_=wv_in.ap().rearrange("(t p) d -> p t d", p=128))
            nc.sync.dma_start(
                out=bq_sb, in_=bq_in.ap().rearrange("(m p) o -> p m o", p=128))
            nc.sync.dma_start(
                out=bk_sb, in_=bk_in.ap().rearrange("(m p) o -> p m o", p=128))
            # broadcast bv along partitions (stride-0 partition AP)
            bv_ap = bass.AP(tensor=bv_in, offset=0, ap=[[0, 128], [1, G]])
            nc.gpsimd.dma_start(out=bv_bc, in_=bv_ap)
            # Wo as stacked head pairs: [two*64+p, pr, n]; first needed
            # only at ~60us, so chain it behind the wv load
            wo_dma = nc.sync.dma_start(
                out=wo_pr,
                in_=wo_in.ap().rearrange("(pr two) p n -> (two p) pr n", two=2))
            add_dep_helper(wo_dma.ins, x_dmas[3].ins,
                           reason="wo after x3: keep first wave fast")
            # ones columns of V'
            nc.gpsimd.memset(vp[:, :, :, 64:65], 1.0)
            nc.gpsimd.memset(ones64, 1.0)

            # ---- warmup: keep the PE busy through the DMA lead-in ----
            # sized to END when wk/x0 land (~16.5us): the PE runs at the
            # pre-HAM half clock (~0.9GHz) here, so ~7k column-cycles.
            for wi in range(14):
                ps_d = ps_op_pool.tile([128, 512], f32, tag="dummy",
                                       name=f"warm_{wi}")
                nc.tensor.matmul(ps_d, lhsT=warm[:, 0:128], rhs=warm,
                                 start=True, stop=True)

            # ---- projection building blocks (used as fillers) ----
            # alternate PSUM tags so back-to-back fillers land in
            # different banks and don't serialize on the DVE evacuation
            _ftag = ["dummy"]

            def next_ftag():
                _ftag[0] = "oproj" if _ftag[0] == "dummy" else "dummy"
                return _ftag[0]

            def qk_half(w_sb, b_sb, dst, jc, m, half, st):
                sl = slice(jc * 512, (jc + 1) * 512)
                if half == 0:
                    st["ps"] = ps_op_pool.tile(
                        [128, 512], f32, tag=next_ftag(),
                        name=f"psqk_{id(w_sb)}_{jc}_{m}")
                for ht in range(half * 4, half * 4 + 4):
                    nc.tensor.matmul(
                        st["ps"],
                        lhsT=w_sb[:, ht, m * 128:(m + 1) * 128],
                        rhs=xTc[jc][:, ht, :],
                        start=(ht == 0), stop=(ht == 7))
                if half == 1:
                    nc.vector.tensor_scalar_add(
                        dst[:, m, sl], st["ps"], b_sb[:, m, :])

            def v_unit(st16):
                ps_vt = ps_op_pool.tile([128, 512], f32, tag=next_ftag(),
                                        name=f"psv_{st16}")
                for ht in range(8):
                    nc.tensor.matmul(
                        ps_vt[:, 0:G],
                        lhsT=xTc[st16 // 4][:, ht,
                                            (st16 % 4) * 128:
                                            (st16 % 4 + 1) * 128],
                        rhs=wv_sb[:, ht, :],
                        start=(ht == 0), stop=(ht == 7))
                nc.vector.tensor_add(
                    vp[:, st16, :, 0:64],
                    ps_vt[:, 0:G].rearrange("p (h d) -> p h d", h=NHL),
                    bv_bc.rearrange("p (h d) -> p h d", h=NHL))

            # ---- filler queue: (deadline_slot, cycles, closure) ----
            fillers = []

            def add_qk(w_sb, b_sb, dst, jc, m, deadline):
                st = {}
                fillers.append((deadline, 2048, lambda: qk_half(
                    w_sb, b_sb, dst, jc, m, 0, st)))
                fillers.append((deadline, 2048, lambda: qk_half(
                    w_sb, b_sb, dst, jc, m, 1, st)))

            # V st: needed by AV(st) issued at slot st+1
            for st16 in range(16):
                fillers.append((st16 + 1, 2048,
                                lambda s=st16: v_unit(s)))
            # K m0 jc1-3: needed by S(slot 4*jc)
            for jc in range(1, 4):
                add_qk(wk_sb, bk_sb, kT, jc, 0, 4 * jc)
            # K m1: needed by S of unit 1 (slots 16+4*jc)
            for jc in range(4):
                add_qk(wk_sb, bk_sb, kT, jc, 1, 16 + 4 * jc)
            # Q c0 m1: needed at slot 16
            add_qk(wq_sb, bq_sb, qT, 0, 1, 14)
            # Q c1-3 m0/m1: pulled a few slots ahead of their unit start
            # so they interleave with S/AV instead of bursting at the
            # boundary (where the DVE is busy with norm/oproj evacs)
            for qc in range(1, 4):
                add_qk(wq_sb, bq_sb, qT, qc, 0, 32 * qc - 6)
                add_qk(wq_sb, bq_sb, qT, qc, 1, 32 * qc + 6)
            fillers.sort(key=lambda f: f[0])
            total_fill = sum(f[1] for f in fillers)
            fill_issued = [0]

            def run_filler():
                _, cyc, fn = fillers.pop(0)
                fn()
                fill_issued[0] += cyc

            def dummy(n):
                ps_d = ps_op_pool.tile([128, 512], f32, tag="dummy")
                nc.tensor.matmul(ps_d[:, 0:n], lhsT=kT[:, 0, 0:128],
                                 rhs=qT[:, 0, 0:n], start=True, stop=True)

            def norm_head(outP, ps_av, hh, qc, mt, tail=False):
                uout = tmpo_pool.tile([HD, 512], f32, tag="uout",
                                      name=f"uo_{qc}_{mt}_{hh}", bufs=4)
                sums = sums_pool.tile([1, 512], f32, tag="sums",
                                      name=f"sm_{qc}_{mt}_{hh}")
                if tail:
                    # latency-critical: reciprocal chain straight from
                    # the PSUM sums row, uout evacuation deferred
                    nc.vector.tensor_copy(sums, ps_av[64:65, :])
                else:
                    # bank-release-critical: evacuate PSUM first so the
                    # next unit's AV stream can start
                    nc.vector.tensor_copy(uout, ps_av)
                    nc.vector.tensor_copy(sums, uout[64:65, :])
                recip = sums_pool.tile([1, 512], f32, tag="recip",
                                       name=f"rc_{qc}_{mt}_{hh}")
                nc.vector.reciprocal_approx_fast(out=recip, in_=sums)
                recip_bf = sums_pool.tile([1, 512], bf16, tag="recipb",
                                          name=f"rcb_{qc}_{mt}_{hh}")
                nc.vector.tensor_copy(recip_bf, recip)
                if tail:
                    nc.vector.tensor_copy(uout, ps_av)
                # broadcast along partitions: rank-1 outer product on
                # the PE (ones[1,64].T @ recip[1,512] -> [64,512])
                rbc = ps_op_pool.tile([64, 512], f32, tag="dummy",
                                      name=f"rb_{qc}_{mt}_{hh}")
                nc.tensor.matmul(rbc, lhsT=ones64, rhs=recip_bf,
                                 start=True, stop=True)
                nc.vector.tensor_mul(
                    outP[hh * 64:hh * 64 + 64, :], uout[0:64, :], rbc)

            def oproj_unit(qc, outPs, qt, tail=False):
                # out_proj for one q-tile (K=128 stacked pairs); at the
                # kernel tail the freed score slots double-buffer it and
                # each half is DMA'd as soon as it is evacuated
                osb = osb_pool.tile([128, H], bf16, tag="osb",
                                    name=f"osb_{qc}_{qt}")
                r0 = qc * 512 + qt * 128
                for ncx in range(2):
                    if tail:
                        ps_op = ps_s_pool.tile(
                            [128, 2, 512], f32, tag="s",
                            name=f"psot_{qc}_{qt}_{ncx}")[:, 0, :]
                    else:
                        ps_op = ps_op_pool.tile(
                            [128, 512], f32, tag="oproj",
                            name=f"pso_{qc}_{qt}_{ncx}")
                    for pr in range(2):
                        nc.tensor.matmul(
                            ps_op,
                            lhsT=outPs[pr][:, qt * 128:(qt + 1) * 128],
                            rhs=wo_pr[:, pr, ncx * 512:(ncx + 1) * 512],
                            start=(pr == 0), stop=(pr == 1))
                    nc.vector.tensor_copy(
                        osb[:, ncx * 512:(ncx + 1) * 512], ps_op)
                # full rows are contiguous in DRAM (2KB bursts)
                nc.sync.dma_start(out=out_d.ap()[r0:r0 + 128, :],
                                  in_=osb)

            # ---- prologue: minimal pre-score critical path ----
            stp = {}
            qk_half(wk_sb, bk_sb, kT, 0, 0, 0, stp)
            qk_half(wk_sb, bk_sb, kT, 0, 0, 1, stp)
            stp = {}
            qk_half(wq_sb, bq_sb, qT, 0, 0, 0, stp)
            qk_half(wq_sb, bq_sb, qT, 0, 0, 1, stp)

            # ---- master attention loop: 8 units x 16 kt slots ----
            UNITS = [(qc, mt) for qc in range(4) for mt in range(2)]
            pend_av = None       # (attnT, ps_avs, qc, mt, kt) awaiting AV
            pend_norm = None     # (qc, mt, ps_avs) awaiting normalize
            pend_oproj = []      # oproj closures, drained one per window
            outP_by_qc = {}
            held_q2 = []         # qc=2 outP pair held for the tail

            def issue_av(p):
                at_t, avs, p_qc, p_mt, p_kt = p
                for hh in range(2):
                    nc.tensor.matmul(
                        avs[hh],
                        lhsT=vp[:, p_kt, 2 * p_mt + hh, :],
                        rhs=at_t[:, hh, p_kt % 4, :],
                        start=(p_kt == 0), stop=(p_kt == 15))
                return (p_qc, p_mt, avs) if p_kt == 15 else None

            def do_norm(p_qc, p_mt, avs, norm_slot=0, last=False):
                outP = op_pool.tile([128, 512], bf16, tag="outP",
                                    name=f"outP_{p_qc}_{p_mt}")
                for hh in range(2):
                    norm_head(outP, avs[hh], hh, p_qc, p_mt, tail=last)
                outP_by_qc.setdefault(p_qc, []).append(outP)
                if p_mt == 1 and not last:
                    pouts = outP_by_qc.pop(p_qc)
                    n_inloop = 4
                    if p_qc == 2:
                        held_q2.append(pouts)
                    for qt in range(n_inloop):
                        # defer the drain so half the out_proj work lands
                        # in the following (ACT-paced) unit
                        pend_oproj.append(
                            (norm_slot + 8,
                             lambda q=p_qc, o=pouts, t=qt:
                             oproj_unit(q, o, t)))

            for s in range(128):
                u, kt = s // 16, s % 16
                qc, mt = UNITS[u]
                qsl = slice(qc * 512, (qc + 1) * 512)

                if kt == 0:
                    attnT = at_pool.tile([128, 2, 4, 512], bf16,
                                         tag="at", name=f"at_{qc}_{mt}")
                    ps_avs = [ps_av_pool.tile([HD, 512], f32, tag="av",
                                              name=f"av_{qc}_{mt}_{hh}")
                              for hh in range(2)]

                # forced fillers: everything whose deadline has arrived
                while fillers and fillers[0][0] <= s:
                    run_filler()

                # scores + exp for this slot
                ps_s = ps_s_pool.tile([128, 2, 512], f32, tag="s")
                for hh in range(2):
                    nc.tensor.matmul(
                        ps_s[:, hh, :],
                        lhsT=kT[hh * 64:hh * 64 + 64, mt,
                                kt * 128:(kt + 1) * 128],
                        rhs=qT[hh * 64:hh * 64 + 64, mt, qsl],
                        start=True, stop=True)
                nc.scalar.activation(
                    out=attnT[:, :, kt % 4, :], in_=ps_s, func=EXP)

                # lagged AV from the previous slot; when it closes a
                # unit (kt==15), queue that unit's normalize
                if pend_av is not None:
                    done = issue_av(pend_av)
                    if done is not None:
                        pend_norm = done
                pend_av = (attnT, ps_avs, qc, mt, kt)

                # normalize the unit whose AV stream just closed
                if pend_norm is not None and kt == 1:
                    p_qc, p_mt, p_avs2 = pend_norm
                    do_norm(p_qc, p_mt, p_avs2, norm_slot=s)
                    pend_norm = None

                # out_proj: one q-tile per 4-slot window once eligible
                if pend_oproj and pend_oproj[0][0] <= s and kt % 4 == 2:
                    pend_oproj.pop(0)[1]()

                # paced optional fillers: keep the stream carrying real
                # work end-to-end instead of front-loading
                while (fillers and
                       fill_issued[0] * 116 < total_fill * (s + 1)):
                    run_filler()

                if not fillers and not pend_oproj and kt % 4 == 3:
                    dummy(256)

            # ---- tail: AV(15) of last unit, final norm, oproj ----
            # hand-staged engine ordering: the DVE reciprocal chains are
            # issued first so nothing delays them; the held qc=2 out_proj
            # matmuls keep the PE dense (HAM at full clock) while the
            # normalize chain drains; evacuations come after.
            while pend_oproj:
                pend_oproj.pop(0)[1]()
            for _ in range(2):
                dummy(512)
            p_qc, p_mt, avs = issue_av(pend_av)

            # ACT (idle after the last exp): copy the PSUM sums rows out
            # so the DVE reciprocals can start without a DVE-side copy
            sums_t, recips, uouts = [], [], []
            for hh in range(2):
                s_t = sums_pool.tile([1, 512], f32, tag="sums",
                                     name=f"sm_t_{hh}")
                nc.scalar.activation(
                    out=s_t, in_=avs[hh][64:65, :],
                    func=mybir.ActivationFunctionType.Copy)
                sums_t.append(s_t)

            # DVE: uout evacuations then reciprocals (from the ACT
            # copies); GPSIMD: the bf16 casts
            for hh in range(2):
                uout = tmpo_pool.tile([HD, 512], f32, tag="uout",
                                      name=f"uo_t_{hh}", bufs=4)
                nc.vector.tensor_copy(uout, avs[hh])
                uouts.append(uout)
            for hh in range(2):
                recip = sums_pool.tile([1, 512], f32, tag="recip",
                                       name=f"rc_t_{hh}")
                nc.vector.reciprocal_approx_fast(out=recip,
                                                 in_=sums_t[hh])
                recips.append(recip)
            recips_bf = []
            for hh in range(2):
                rbf = sums_pool.tile([1, 512], bf16, tag="recipb",
                                     name=f"rcb_t_{hh}")
                nc.gpsimd.tensor_copy(rbf, recips[hh])
                recips_bf.append(rbf)

            # PE: dummy cover while the chains drain, then the
            # reciprocal broadcasts; DVE: normalize multiplies
            outP = op_pool.tile([128, 512], bf16, tag="outP",
                                name="outP_3_1")
            for _ in range(8):
                dummy(512)
            rbcs = []
            for hh in range(2):
                rbc = ps_op_pool.tile(
                    [64, 512], f32, tag="dummy" if hh == 0 else "oproj",
                    name=f"rb_t_{hh}")
                nc.tensor.matmul(rbc, lhsT=ones64, rhs=recips_bf[hh],
                                 start=True, stop=True)
                rbcs.append(rbc)
            for hh in range(2):
                nc.vector.tensor_mul(
                    outP[hh * 64:hh * 64 + 64, :], uouts[hh][0:64, :],
                    rbcs[hh])
            outP_by_qc.setdefault(3, []).append(outP)

            # final out_proj for qc=3; PSUM tags chosen so qt0 does not
            # wait on the held-qt3 score-slot evacuations
            pouts = outP_by_qc.pop(3)
            for qt in range(4):
                osb = osb_pool.tile([128, H], bf16, tag="osb",
                                    name=f"osb_3_{qt}")
                for ncx in range(2):
                    if qt in (0, 3):
                        ps_op = ps_op_pool.tile(
                            [128, 512], f32,
                            tag="oproj" if ncx == 0 else "dummy",
                            name=f"pso3_{qt}_{ncx}")
                    else:
                        ps_op = ps_s_pool.tile(
                            [128, 2, 512], f32, tag="s",
                            name=f"pso3_{qt}_{ncx}")[:, 0, :]
                    for pr in range(2):
                        nc.tensor.matmul(
                            ps_op,
                            lhsT=pouts[pr][:, qt * 128:(qt + 1) * 128],
                            rhs=wo_pr[:, pr, ncx * 512:(ncx + 1) * 512],
                            start=(pr == 0), stop=(pr == 1))
                    nc.vector.tensor_copy(
                        osb[:, ncx * 512:(ncx + 1) * 512], ps_op)
                nc.sync.dma_start(
                    out=out_d.ap()[3 * 512 + qt * 128:
                                   3 * 512 + (qt + 1) * 128, :],
                    in_=osb)

    nc.compile()
    _CACHE["nc"] = nc
    return nc


def make_in_maps(x, Wq, bq, Wk, bk, Wv, bv, Wo):
    import ml_dtypes
    bf = ml_dtypes.bfloat16

    x = np.asarray(x, dtype=np.float32)
    Wq = np.asarray(Wq, dtype=np.float32)
    bq = np.asarray(bq, dtype=np.float32)
    Wk = np.asarray(Wk, dtype=np.float32)
    bk = np.asarray(bk, dtype=np.float32)
    Wv = np.asarray(Wv, dtype=np.float32)
    bv = np.asarray(bv, dtype=np.float32)
    Wo = np.asarray(Wo, dtype=np.float32)

    scale = np.float32(1.0 / 8.0)  # 1/sqrt(64)

    in_maps = []
    for core in range(N_CORES):
        b = core // 4
        g = core % 4
        cs = slice(g * G, (g + 1) * G)
        in_maps.append({
            "xt": np.ascontiguousarray(
                x[b].reshape(4, 512, 8, 128).transpose(0, 3, 2, 1)).astype(bf),
            "wq": np.ascontiguousarray(Wq[:, cs] * scale).astype(bf),
            "wk": np.ascontiguousarray(Wk[:, cs]).astype(bf),
            "wv": np.ascontiguousarray(Wv[:, cs]).astype(bf),
            "bq": np.ascontiguousarray((bq[cs] * scale).reshape(G, 1)),
            "bk": np.ascontiguousarray(bk[cs].reshape(G, 1)),
            "bv": np.ascontiguousarray(bv[cs]),
            "wo": np.ascontiguousarray(Wo[cs, :].reshape(NHL, 64, H)).astype(bf),
        })
    return in_maps


def kernel(x, Wq, bq, Wk, bk, Wv, bv, Wo, bo):
    from concourse.bass_utils import run_bass_kernel_spmd

    bo = np.asarray(bo, dtype=np.float32)
    nc = _build()
    in_maps = make_in_maps(x, Wq, bq, Wk, bk, Wv, bv, Wo)
    res = run_bass_kernel_spmd(nc, in_maps, core_ids=list(range(N_CORES)))

    out = np.empty((2, S, H), dtype=np.float32)
    for b in range(2):
        acc = res.results[4 * b]["out"].astype(np.float32)
        for g in range(1, 4):
            acc = acc + res.results[4 * b + g]["out"].astype(np.float32)
        out[b] = acc + bo
    return out
